# revision 2
# baseline (speedup 1.0000x reference)
"""Trainium2 Bass kernel for PooledSelfAttention2d — 16-bit pipeline.

Derived from the f32r baseline schedule, with:
  - x loaded ONCE as f16 (8MB/core instead of 32MB): feeds the theta/phi
    conv, the g conv, and stays resident in SBUF for the residual add.
  - all matmul operands f16 (convs, logits, out conv) or bf16 (ex, gT —
    bf16 because ex = exp(logits - 60) spans e^[-55, 29], beyond f16 range).
  - transposes in 16-bit (1.0 cycles/row vs 1.5 for f32r).
  - y stored as f16 (8MB/core instead of 16MB), upcast on host.
  - g path pools PSUM directly (pool-first), relu+bias after the 4-way max.
  - exp/relu/copy work spread across Act, DVE, and GPSIMD; residual adds
    hit the DVE 2x mode (all-f16 operands).

Softmax uses the baseline's constant -60 shift (logits lie in [5, 89] for
this input distribution; bf16 has fp32's exponent range so no per-row max
is needed; softmax is invariant to the shift).
"""

import sys

if "/opt/trn_rl_repo" not in sys.path:
    sys.path.insert(0, "/opt/trn_rl_repo")

import contextlib

import ml_dtypes
import numpy as np

import concourse.bacc as bacc
import concourse.bass as bass
import concourse.tile as tile
from concourse import mybir
from concourse.bass_utils import run_bass_kernel_spmd

F32 = mybir.dt.float32
F16 = mybir.dt.float16
BF16 = mybir.dt.bfloat16
AF = mybir.ActivationFunctionType
ALU = mybir.AluOpType

B, C, H, W = 16, 512, 64, 64
N = H * W            # 4096 pixels
M = N // 4           # 1024 pooled pixels
K8 = C // 8          # 64  (theta/phi channels)
C2 = C // 2          # 256 (g channels)
NCORES = 8
BPC = B // NCORES    # batches per core
NT = N // 512        # n-tiles of 512 pixels
EXP_SHIFT = -60.0    # constant softmax shift


def _build_program():
    nc = bacc.Bacc("TRN2", target_bir_lowering=False, debug=False)

    x_h = nc.dram_tensor("x16", [BPC, C, N], F16, kind="ExternalInput").ap()
    wtp_h = nc.dram_tensor("wtp16", [128, 4, 128], F16, kind="ExternalInput").ap()
    wg_h = nc.dram_tensor("wg16", [128, 4, C2], F16, kind="ExternalInput").ap()
    wo_h = nc.dram_tensor("wo16", [128, 2, C], F16, kind="ExternalInput").ap()
    bia_h = nc.dram_tensor("biases", [128, 8], F32, kind="ExternalInput").ap()
    idn_h = nc.dram_tensor("idn16", [128, 128], F16, kind="ExternalInput").ap()
    idb_h = nc.dram_tensor("idnb", [128, 128], BF16, kind="ExternalInput").ap()
    y_h = nc.dram_tensor("y", [BPC, C, N], F16, kind="ExternalOutput").ap()

    xv = x_h.rearrange("b (cc p) n -> b p cc n", p=128)
    yv = y_h.rearrange("b (cc p) n -> b p cc n", p=128)

    with tile.TileContext(nc) as tc:
        with contextlib.ExitStack() as ctx:
            consts = ctx.enter_context(tc.tile_pool(name="consts", bufs=1))
            bpool = ctx.enter_context(tc.tile_pool(name="bpool", bufs=2))
            xpool = ctx.enter_context(tc.tile_pool(name="xpool", bufs=1))
            work = ctx.enter_context(tc.tile_pool(name="work", bufs=2))
            psum = ctx.enter_context(tc.tile_pool(name="psum", bufs=1, space="PSUM"))

            # ---- first x tile prefetch, then constants ----
            xr0 = xpool.tile([128, 4, 512], F16, tag="xr", bufs=NT * 2 + 1)
            nc.gpsimd.dma_start(out=xr0, in_=xv[0, :, :, 0:512])

            wtp_sb = consts.tile([128, 4, 128], F16, tag="wtp")
            nc.sync.dma_start(out=wtp_sb, in_=wtp_h)
            wg_sb = consts.tile([128, 4, C2], F16, tag="wg")
            nc.sync.dma_start(out=wg_sb, in_=wg_h)
            bia_sb = consts.tile([128, 8], F32, tag="bia")
            nc.sync.dma_start(out=bia_sb, in_=bia_h)
            wo_sb = consts.tile([128, 2, C], F16, tag="wo")
            idn_sb = consts.tile([128, 128], F16, tag="idn")
            idb_sb = consts.tile([128, 128], BF16, tag="idb")
            shift_sb = consts.tile([128, 1], F32, tag="shift")
            nc.vector.memset(shift_sb, EXP_SHIFT)
            onez_sb = consts.tile([128, 2], BF16, tag="onez")
            nc.vector.memset(onez_sb, 0.0)
            nc.vector.memset(onez_sb[:, 0:1], 1.0)

            # per-batch persistent tiles
            def batch_tiles(b):
                tp = bpool.tile([128, N], F16, tag="tp", name=f"tp{b}")
                phip = bpool.tile([64, M], F16, tag="phip", name=f"phip{b}")
                gp = bpool.tile([128, 2, M], BF16, tag="gp", name=f"gp{b}")
                gT = bpool.tile([128, 8, C2 + 2], BF16, tag="gT", name=f"gT{b}")
                onez_b = bass.AP(
                    tensor=onez_sb.tensor,
                    offset=onez_sb.offset,
                    ap=[list(onez_sb.ap[0]), [0, 8], list(onez_sb.ap[1])],
                )
                nc.vector.tensor_copy(gT[:, :, C2:C2 + 2], onez_b)
                return dict(tp=tp, phip=phip, gp=gp, gT=gT, xr=[None] * NT)

            def emit_A_load(S, b, i):
                nsl = slice(i * 512, (i + 1) * 512)
                if b == 0 and i == 0:
                    S["xr"][i] = xr0
                    return xr0
                xr = xpool.tile([128, 4, 512], F16, tag="xr", bufs=NT * 2 + 1,
                                name=f"xr{b}_{i}")
                eng = nc.gpsimd if i % 2 == 0 else nc.scalar
                eng.dma_start(out=xr, in_=xv[b, :, :, nsl])
                S["xr"][i] = xr
                return xr

            def emit_A_tile(S, b, i, xr):
                nsl = slice(i * 512, (i + 1) * 512)
                msl = slice(i * 128, (i + 1) * 128)
                psTP = psum.tile([128, 512], F32, tag="mmL", bufs=4)
                for cc in range(4):
                    nc.tensor.matmul(psTP, wtp_sb[:, cc, :], xr[:, cc, :],
                                     start=(cc == 0), stop=(cc == 3))
                # theta+phi relu+bias evac (Act)
                nc.scalar.activation(S["tp"][:, nsl], psTP, AF.Relu,
                                     bias=bia_sb[:, 0:1])
                for oc in range(2):
                    psG = psum.tile([128, 512], F32, tag="mmL", bufs=4,
                                    name=f"psG{oc}")
                    for cc in range(4):
                        nc.tensor.matmul(
                            psG, wg_sb[:, cc, oc * 128:(oc + 1) * 128],
                            xr[:, cc, :], start=(cc == 0), stop=(cc == 3))
                    # relu+bias evac (Act; gpsimd cannot read PSUM), then
                    # 2x2 maxpool on SBUF (gpsimd)
                    gf = work.tile([128, 512], BF16, tag="gf", bufs=3,
                                   name=f"gf{oc}")
                    nc.scalar.activation(gf, psG, AF.Relu,
                                         bias=bia_sb[:, 1 + oc:2 + oc])
                    gv = gf.rearrange("p (a two) -> p a two", two=2)
                    gw = work.tile([128, 256], BF16, tag="gw", bufs=4,
                                   name=f"gw{oc}")
                    nc.vector.tensor_max(gw, gv[:, :, 0], gv[:, :, 1])
                    gw2 = gw.rearrange("p (h two w) -> p h two w", two=2, w=32)
                    gdst = S["gp"][:, oc, msl].rearrange("p (h w) -> p h w", w=32)
                    nc.vector.tensor_max(gdst, gw2[:, :, 0, :], gw2[:, :, 1, :])
                # phi pool (gpsimd, SBUF) from tp rows 64:128
                phv = S["tp"][64:128, nsl].rearrange("p (a two) -> p a two",
                                                     two=2)
                phw = work.tile([64, 256], F16, tag="phw", bufs=2)
                nc.vector.tensor_max(phw, phv[:, :, 0], phv[:, :, 1])
                phw2 = phw.rearrange("p (h two w) -> p h two w", two=2, w=32)
                pdst = S["phip"][:, msl].rearrange("p (h w) -> p h w", w=32)
                nc.vector.tensor_max(pdst, phw2[:, :, 0, :], phw2[:, :, 1, :])

            def emit_B_chunk(S, mi):
                """transpose pooled g chunk mi to gT (bf16)."""
                msl = slice(mi * 128, (mi + 1) * 128)
                psT = psum.tile([128, C2], BF16, tag="mmL", bufs=4,
                                name=f"psT{mi}")
                nc.tensor.transpose(psT[:, 0:128], S["gp"][:, 0, msl], idb_sb)
                nc.tensor.transpose(psT[:, 128:256], S["gp"][:, 1, msl], idb_sb)
                nc.scalar.copy(S["gT"][:, mi, 0:C2], psT)

            def emit_L(S, cur, k):
                """logits for m-chunk k + exp -> bf16."""
                psL = psum.tile([128, 512], F32, tag="mmL", bufs=4)
                nc.tensor.matmul(psL, S["phip"][:, k * 128:(k + 1) * 128],
                                 S["tp"][0:64, cur["nsl"]], start=True,
                                 stop=True)
                ex = work.tile([128, 512], BF16, tag="exp", bufs=4)
                nc.scalar.activation(ex, psL, AF.Exp, bias=shift_sb)
                cur["ex"].append(ex)

            def emit_bmm(S, cur, k, norm_tail=False):
                for ns in range(4):
                    ssl = slice(ns * 128, (ns + 1) * 128)
                    nc.tensor.matmul(
                        cur["psOp"][ns // 2][:, ns % 2, 0:C2 + 2],
                        cur["ex"][k][:, ssl], S["gT"][:, k, :],
                        start=(k == 0), stop=(k == 7))
                    if norm_tail:
                        emit_norm_ns(cur, ns)

            def emit_norm_ns(prev, ns):
                """normalize one n-sub of tile i-1 (DVE only) -> f16."""
                psO = prev["psOp"][ns // 2][:, ns % 2, :]
                rec = work.tile([128, 1], F32, tag="rec", bufs=8)
                nc.vector.reciprocal(rec, psO[:, C2:C2 + 1])
                onc = work.tile([128, C2], F16, tag="onc", bufs=5,
                                name=f"onc{ns}")
                nc.vector.tensor_scalar_mul(onc, psO[:, 0:C2], rec)
                prev.setdefault("onc", []).append(onc)

            def emit_transp(prev):
                """transpose tile i-1 back to channel-major (PE, f16)."""
                prev["pst"] = [
                    psum.tile([128, 512], F16, tag="mmL", bufs=4,
                              name=f"pst{c2}")
                    for c2 in range(2)
                ]
                for ns in range(4):
                    ssl = slice(ns * 128, (ns + 1) * 128)
                    onc = prev["onc"][ns]
                    for c2 in range(2):
                        nc.tensor.transpose(
                            prev["pst"][c2][:, ssl],
                            onc[:, c2 * 128:(c2 + 1) * 128], idn_sb)

            def emit_final_start(prev):
                """evacuate transposed o to SBUF (tile i-1), f16."""
                ocm = [
                    work.tile([128, 512], F16, tag="ocm", bufs=3,
                              name=f"ocm{j}")
                    for j in range(2)
                ]
                nc.vector.tensor_copy(ocm[0], prev["pst"][0])
                nc.vector.tensor_copy(ocm[1], prev["pst"][1])
                prev["ocm"] = ocm
                prev["yt"] = work.tile([128, 4, 512], F16, tag="y", bufs=2,
                                       name="yt")

            def emit_final_oc(S, prev, oc):
                """one output-conv chunk + relu+bias + residual (+ store)."""
                ocm, yt = prev["ocm"], prev["yt"]
                psY = psum.tile([128, 512], F32, tag="mmL", bufs=4)
                nc.tensor.matmul(psY, wo_sb[:, 0, oc * 128:(oc + 1) * 128],
                                 ocm[0], start=True, stop=False)
                nc.tensor.matmul(psY, wo_sb[:, 1, oc * 128:(oc + 1) * 128],
                                 ocm[1], start=False, stop=True)
                # y = relu(conv + b) + x   (relu+bias split across engines)
                if oc % 2 == 1:
                    nc.scalar.activation(yt[:, oc, :], psY, AF.Relu,
                                         bias=bia_sb[:, 3 + oc:4 + oc])
                else:
                    nc.vector.tensor_scalar(yt[:, oc, :], psY,
                                            bia_sb[:, 3 + oc:4 + oc], 0.0,
                                            ALU.add, ALU.max)
                xr = S["xr"][prev["i"]]
                nc.vector.tensor_add(yt[:, oc, :], yt[:, oc, :], xr[:, oc, :])
                if oc % 2 == 1:
                    nc.sync.dma_start(
                        out=yv[prev["b"], :, oc - 1:oc + 1, prev["nsl"]],
                        in_=yt[:, oc - 1:oc + 1, :])

            # ================= main schedule =================
            S = {0: batch_tiles(0)}
            nxt = None
            prev = None
            _xq = [emit_A_load(S[0], 0, 0)]
            for i in range(NT):
                if i + 1 < NT:
                    _xq.append(emit_A_load(S[0], 0, i + 1))
                emit_A_tile(S[0], 0, i, _xq.pop(0))
                if i == 1:
                    nc.sync.dma_start(out=wo_sb, in_=wo_h)
                    nc.sync.dma_start(out=idn_sb, in_=idn_h)
                    nc.sync.dma_start(out=idb_sb, in_=idb_h)
            for b in range(BPC):
                if b + 1 < BPC:
                    S[b + 1] = batch_tiles(b + 1)

                def make_cur(bb, i):
                    nsl = slice(i * 512, (i + 1) * 512)
                    cur = {"nsl": nsl, "b": bb, "i": i, "ex": []}
                    cur["psOp"] = [
                        psum.tile([128, 2, 512], F32, tag="obmm", bufs=2,
                                  name=f"psOp{j}")
                        for j in range(2)
                    ]
                    # cross-batch safe: chunks 0/1 of phip are pooled by the
                    # A-phase of tiles 0/1, long since emitted
                    emit_L(S[bb], cur, 0)
                    emit_L(S[bb], cur, 1)
                    return cur

                for i in range(NT):
                    cur = nxt if nxt is not None else make_cur(b, i)
                    nxt = None
                    if i == 0:
                        # gT transposes for this batch (A fully emitted);
                        # interleaved into the k-loop below
                        emit_B_chunk(S[b], 0)
                        emit_B_chunk(S[b], 1)
                    emit_L(S[b], cur, 2)
                    emit_L(S[b], cur, 3)
                    if prev is not None:
                        # norms were interleaved with prev's k==7 bmm
                        emit_transp(prev)
                        emit_final_start(prev)
                    for k in range(8):
                        if i == 0 and k < 6:
                            emit_B_chunk(S[b], k + 2)
                        emit_bmm(S[b], cur, k, norm_tail=(k == 7))
                        if prev is not None and 1 <= k <= 4:
                            emit_final_oc(S[prev["b"]], prev, k - 1)
                        if k == 0:
                            emit_L(S[b], cur, 4)
                            emit_L(S[b], cur, 5)
                        if k == 2:
                            emit_L(S[b], cur, 6)
                            emit_L(S[b], cur, 7)
                        if k == 6:
                            if i + 1 < NT:
                                nxt = make_cur(b, i + 1)
                            elif b + 1 < BPC:
                                nxt = make_cur(b + 1, 0)

                    prev = cur
                    if b + 1 < BPC:
                        if i == 0:
                            _xq.append(emit_A_load(S[b + 1], b + 1, 0))
                        if i + 1 < NT:
                            _xq.append(emit_A_load(S[b + 1], b + 1, i + 1))
                        emit_A_tile(S[b + 1], b + 1, i, _xq.pop(0))
                # drain the last tile of the batch (norms already emitted
                # with its k==7 bmm)
                emit_transp(prev)
                emit_final_start(prev)
                for oc in range(4):
                    emit_final_oc(S[prev["b"]], prev, oc)

    nc.compile()
    return nc


_CACHE = {}


def _get_program():
    if "nc" not in _CACHE:
        _CACHE["nc"] = _build_program()
    return _CACHE["nc"]


def prepare_in_maps(inputs):
    x = np.ascontiguousarray(inputs["x"], dtype=np.float32).reshape(B, C, N)
    W_theta = np.asarray(inputs["W_theta"], dtype=np.float32)
    b_theta = np.asarray(inputs["b_theta"], dtype=np.float32)
    W_phi = np.asarray(inputs["W_phi"], dtype=np.float32)
    b_phi = np.asarray(inputs["b_phi"], dtype=np.float32)
    W_g = np.asarray(inputs["W_g"], dtype=np.float32)
    b_g = np.asarray(inputs["b_g"], dtype=np.float32)
    W_o = np.asarray(inputs["W_o"], dtype=np.float32)
    b_o = np.asarray(inputs["b_o"], dtype=np.float32)
    gamma = float(np.asarray(inputs["gamma"]).reshape(-1)[0])

    x16 = x.astype(np.float16)
    wtp16 = np.ascontiguousarray(
        np.concatenate([W_theta, W_phi], axis=0).T.reshape(4, 128, 128)
        .transpose(1, 0, 2)).astype(np.float16)
    wg16 = np.ascontiguousarray(
        W_g.T.reshape(4, 128, C2).transpose(1, 0, 2)).astype(np.float16)
    wo16 = np.ascontiguousarray(
        (gamma * W_o).T.reshape(2, 128, C).transpose(1, 0, 2)).astype(np.float16)
    idn16 = np.eye(128, dtype=np.float16)
    idnb = np.eye(128, dtype=np.float32).astype(ml_dtypes.bfloat16)

    biases = np.zeros((128, 8), np.float32)
    biases[0:64, 0] = b_theta
    biases[64:128, 0] = b_phi
    biases[:, 1] = b_g[0:128]
    biases[:, 2] = b_g[128:256]
    for oc in range(4):
        biases[:, 3 + oc] = gamma * b_o[oc * 128:(oc + 1) * 128]

    shared = {"wtp16": wtp16, "wg16": wg16, "wo16": wo16, "biases": biases,
              "idn16": idn16, "idnb": idnb}
    return [
        {"x16": np.ascontiguousarray(x16[c * BPC:(c + 1) * BPC]), **shared}
        for c in range(NCORES)
    ]


def kernel(**inputs) -> np.ndarray:
    in_maps = prepare_in_maps(inputs)
    nc = _get_program()
    res = run_bass_kernel_spmd(nc, in_maps, core_ids=list(range(NCORES)))
    y = np.concatenate(
        [np.asarray(r["y"]).astype(np.float32) for r in res.results], axis=0)
    return y.reshape(B, C, H, W)


if __name__ == "__main__":
    _get_program()
    print("program built OK")


# revision 3
# speedup vs baseline: 1.0024x; 1.0024x over previous
"""Trainium2 Bass kernel for PooledSelfAttention2d — 16-bit pipeline.

Derived from the f32r baseline schedule, with:
  - x loaded ONCE as f16 (8MB/core instead of 32MB): feeds the theta/phi
    conv, the g conv, and stays resident in SBUF for the residual add.
  - all matmul operands f16 (convs, logits, out conv) or bf16 (ex, gT —
    bf16 because ex = exp(logits - 60) spans e^[-55, 29], beyond f16 range).
  - transposes in 16-bit (1.0 cycles/row vs 1.5 for f32r).
  - y stored as f16 (8MB/core instead of 16MB), upcast on host.
  - g path pools PSUM directly (pool-first), relu+bias after the 4-way max.
  - exp/relu/copy work spread across Act, DVE, and GPSIMD; residual adds
    hit the DVE 2x mode (all-f16 operands).

Softmax uses the baseline's constant -60 shift (logits lie in [5, 89] for
this input distribution; bf16 has fp32's exponent range so no per-row max
is needed; softmax is invariant to the shift).
"""

import sys

if "/opt/trn_rl_repo" not in sys.path:
    sys.path.insert(0, "/opt/trn_rl_repo")

import contextlib

import ml_dtypes
import numpy as np

import concourse.bacc as bacc
import concourse.bass as bass
import concourse.tile as tile
from concourse import mybir
from concourse.bass_utils import run_bass_kernel_spmd

F32 = mybir.dt.float32
F16 = mybir.dt.float16
BF16 = mybir.dt.bfloat16
AF = mybir.ActivationFunctionType
ALU = mybir.AluOpType

B, C, H, W = 16, 512, 64, 64
N = H * W            # 4096 pixels
M = N // 4           # 1024 pooled pixels
K8 = C // 8          # 64  (theta/phi channels)
C2 = C // 2          # 256 (g channels)
NCORES = 8
BPC = B // NCORES    # batches per core
NT = N // 512        # n-tiles of 512 pixels
EXP_SHIFT = -60.0    # constant softmax shift


def _build_program():
    nc = bacc.Bacc("TRN2", target_bir_lowering=False, debug=False)

    x_h = nc.dram_tensor("x16", [BPC, C, N], F16, kind="ExternalInput").ap()
    wtp_h = nc.dram_tensor("wtp16", [128, 4, 128], F16, kind="ExternalInput").ap()
    wg_h = nc.dram_tensor("wg16", [128, 4, C2], F16, kind="ExternalInput").ap()
    wo_h = nc.dram_tensor("wo16", [128, 2, C], F16, kind="ExternalInput").ap()
    bia_h = nc.dram_tensor("biases", [128, 8], F32, kind="ExternalInput").ap()
    idn_h = nc.dram_tensor("idn16", [128, 128], F16, kind="ExternalInput").ap()
    idb_h = nc.dram_tensor("idnb", [128, 128], BF16, kind="ExternalInput").ap()
    y_h = nc.dram_tensor("y", [BPC, C, N], F16, kind="ExternalOutput").ap()

    xv = x_h.rearrange("b (cc p) n -> b p cc n", p=128)
    yv = y_h.rearrange("b (cc p) n -> b p cc n", p=128)

    with tile.TileContext(nc) as tc:
        with contextlib.ExitStack() as ctx:
            consts = ctx.enter_context(tc.tile_pool(name="consts", bufs=1))
            bpool = ctx.enter_context(tc.tile_pool(name="bpool", bufs=2))
            xpool = ctx.enter_context(tc.tile_pool(name="xpool", bufs=1))
            work = ctx.enter_context(tc.tile_pool(name="work", bufs=2))
            psum = ctx.enter_context(tc.tile_pool(name="psum", bufs=1, space="PSUM"))

            # ---- first x tile prefetch, then constants ----
            xr0 = xpool.tile([128, 4, 512], F16, tag="xr", bufs=NT * 2 + 1)
            nc.gpsimd.dma_start(out=xr0, in_=xv[0, :, :, 0:512])

            wtp_sb = consts.tile([128, 4, 128], F16, tag="wtp")
            nc.sync.dma_start(out=wtp_sb, in_=wtp_h)
            wg_sb = consts.tile([128, 4, C2], F16, tag="wg")
            nc.sync.dma_start(out=wg_sb, in_=wg_h)
            bia_sb = consts.tile([128, 8], F32, tag="bia")
            nc.sync.dma_start(out=bia_sb, in_=bia_h)
            wo_sb = consts.tile([128, 2, C], F16, tag="wo")
            idn_sb = consts.tile([128, 128], F16, tag="idn")
            idb_sb = consts.tile([128, 128], BF16, tag="idb")
            shift_sb = consts.tile([128, 1], F32, tag="shift")
            nc.vector.memset(shift_sb, EXP_SHIFT)
            onez_sb = consts.tile([128, 2], BF16, tag="onez")
            nc.vector.memset(onez_sb, 0.0)
            nc.vector.memset(onez_sb[:, 0:1], 1.0)

            # per-batch persistent tiles
            def batch_tiles(b):
                tp = bpool.tile([128, N], F16, tag="tp", name=f"tp{b}")
                phip = bpool.tile([64, M], F16, tag="phip", name=f"phip{b}")
                gp = bpool.tile([128, 2, M], BF16, tag="gp", name=f"gp{b}")
                gT = bpool.tile([128, 8, C2 + 2], BF16, tag="gT", name=f"gT{b}")
                onez_b = bass.AP(
                    tensor=onez_sb.tensor,
                    offset=onez_sb.offset,
                    ap=[list(onez_sb.ap[0]), [0, 8], list(onez_sb.ap[1])],
                )
                nc.vector.tensor_copy(gT[:, :, C2:C2 + 2], onez_b)
                return dict(tp=tp, phip=phip, gp=gp, gT=gT, xr=[None] * NT)

            def emit_A_load(S, b, i):
                nsl = slice(i * 512, (i + 1) * 512)
                if b == 0 and i == 0:
                    S["xr"][i] = xr0
                    return xr0
                xr = xpool.tile([128, 4, 512], F16, tag="xr", bufs=NT * 2 + 1,
                                name=f"xr{b}_{i}")
                eng = nc.gpsimd if i % 2 == 0 else nc.sync
                eng.dma_start(out=xr, in_=xv[b, :, :, nsl])
                S["xr"][i] = xr
                return xr

            def emit_A_tile(S, b, i, xr):
                nsl = slice(i * 512, (i + 1) * 512)
                msl = slice(i * 128, (i + 1) * 128)
                psTP = psum.tile([128, 512], F32, tag="mmL", bufs=4)
                for cc in range(4):
                    nc.tensor.matmul(psTP, wtp_sb[:, cc, :], xr[:, cc, :],
                                     start=(cc == 0), stop=(cc == 3))
                # theta+phi relu+bias evac (Act)
                nc.scalar.activation(S["tp"][:, nsl], psTP, AF.Relu,
                                     bias=bia_sb[:, 0:1])
                for oc in range(2):
                    psG = psum.tile([128, 512], F32, tag="mmL", bufs=4,
                                    name=f"psG{oc}")
                    for cc in range(4):
                        nc.tensor.matmul(
                            psG, wg_sb[:, cc, oc * 128:(oc + 1) * 128],
                            xr[:, cc, :], start=(cc == 0), stop=(cc == 3))
                    # relu+bias evac (Act; gpsimd cannot read PSUM), then
                    # 2x2 maxpool on SBUF (gpsimd)
                    gf = work.tile([128, 512], BF16, tag="gf", bufs=3,
                                   name=f"gf{oc}")
                    nc.scalar.activation(gf, psG, AF.Relu,
                                         bias=bia_sb[:, 1 + oc:2 + oc])
                    gv = gf.rearrange("p (a two) -> p a two", two=2)
                    gw = work.tile([128, 256], BF16, tag="gw", bufs=4,
                                   name=f"gw{oc}")
                    nc.vector.tensor_max(gw, gv[:, :, 0], gv[:, :, 1])
                    gw2 = gw.rearrange("p (h two w) -> p h two w", two=2, w=32)
                    gdst = S["gp"][:, oc, msl].rearrange("p (h w) -> p h w", w=32)
                    nc.vector.tensor_max(gdst, gw2[:, :, 0, :], gw2[:, :, 1, :])
                # phi pool (gpsimd, SBUF) from tp rows 64:128
                phv = S["tp"][64:128, nsl].rearrange("p (a two) -> p a two",
                                                     two=2)
                phw = work.tile([64, 256], F16, tag="phw", bufs=2)
                nc.vector.tensor_max(phw, phv[:, :, 0], phv[:, :, 1])
                phw2 = phw.rearrange("p (h two w) -> p h two w", two=2, w=32)
                pdst = S["phip"][:, msl].rearrange("p (h w) -> p h w", w=32)
                nc.vector.tensor_max(pdst, phw2[:, :, 0, :], phw2[:, :, 1, :])

            def emit_B_chunk(S, mi):
                """transpose pooled g chunk mi to gT (bf16)."""
                msl = slice(mi * 128, (mi + 1) * 128)
                psT = psum.tile([128, C2], BF16, tag="mmL", bufs=4,
                                name=f"psT{mi}")
                nc.tensor.transpose(psT[:, 0:128], S["gp"][:, 0, msl], idb_sb)
                nc.tensor.transpose(psT[:, 128:256], S["gp"][:, 1, msl], idb_sb)
                nc.scalar.copy(S["gT"][:, mi, 0:C2], psT)

            def emit_L(S, cur, k):
                """logits for m-chunk k + exp -> bf16."""
                psL = psum.tile([128, 512], F32, tag="mmL", bufs=4)
                nc.tensor.matmul(psL, S["phip"][:, k * 128:(k + 1) * 128],
                                 S["tp"][0:64, cur["nsl"]], start=True,
                                 stop=True)
                ex = work.tile([128, 512], BF16, tag="exp", bufs=4)
                nc.scalar.activation(ex, psL, AF.Exp, bias=shift_sb)
                cur["ex"].append(ex)

            def emit_bmm(S, cur, k, norm_tail=False):
                for ns in range(4):
                    ssl = slice(ns * 128, (ns + 1) * 128)
                    nc.tensor.matmul(
                        cur["psOp"][ns // 2][:, ns % 2, 0:C2 + 2],
                        cur["ex"][k][:, ssl], S["gT"][:, k, :],
                        start=(k == 0), stop=(k == 7))
                    if norm_tail:
                        emit_norm_ns(cur, ns)

            def emit_norm_ns(prev, ns):
                """normalize one n-sub of tile i-1 (DVE only) -> f16."""
                psO = prev["psOp"][ns // 2][:, ns % 2, :]
                rec = work.tile([128, 1], F32, tag="rec", bufs=8)
                nc.vector.reciprocal(rec, psO[:, C2:C2 + 1])
                onc = work.tile([128, C2], F16, tag="onc", bufs=5,
                                name=f"onc{ns}")
                nc.vector.tensor_scalar_mul(onc, psO[:, 0:C2], rec)
                prev.setdefault("onc", []).append(onc)

            def emit_transp(prev):
                """transpose tile i-1 back to channel-major (PE, f16)."""
                prev["pst"] = [
                    psum.tile([128, 512], F16, tag="mmL", bufs=4,
                              name=f"pst{c2}")
                    for c2 in range(2)
                ]
                for ns in range(4):
                    ssl = slice(ns * 128, (ns + 1) * 128)
                    onc = prev["onc"][ns]
                    for c2 in range(2):
                        nc.tensor.transpose(
                            prev["pst"][c2][:, ssl],
                            onc[:, c2 * 128:(c2 + 1) * 128], idn_sb)

            def emit_final_start(prev):
                """evacuate transposed o to SBUF (tile i-1), f16."""
                ocm = [
                    work.tile([128, 512], F16, tag="ocm", bufs=3,
                              name=f"ocm{j}")
                    for j in range(2)
                ]
                nc.vector.tensor_copy(ocm[0], prev["pst"][0])
                nc.vector.tensor_copy(ocm[1], prev["pst"][1])
                prev["ocm"] = ocm
                prev["yt"] = work.tile([128, 4, 512], F16, tag="y", bufs=2,
                                       name="yt")

            def emit_final_oc(S, prev, oc, drain=False):
                """one output-conv chunk + relu+bias + residual (+ store)."""
                ocm, yt = prev["ocm"], prev["yt"]
                psY = psum.tile([128, 512], F32, tag="mmL", bufs=4)
                nc.tensor.matmul(psY, wo_sb[:, 0, oc * 128:(oc + 1) * 128],
                                 ocm[0], start=True, stop=False)
                nc.tensor.matmul(psY, wo_sb[:, 1, oc * 128:(oc + 1) * 128],
                                 ocm[1], start=False, stop=True)
                # y = relu(conv + b) + x   (relu+bias split across engines)
                if drain or oc % 2 == 1:
                    nc.scalar.activation(yt[:, oc, :], psY, AF.Relu,
                                         bias=bia_sb[:, 3 + oc:4 + oc])
                else:
                    nc.vector.tensor_scalar(yt[:, oc, :], psY,
                                            bia_sb[:, 3 + oc:4 + oc], 0.0,
                                            ALU.add, ALU.max)
                xr = S["xr"][prev["i"]]
                nc.vector.tensor_add(yt[:, oc, :], yt[:, oc, :], xr[:, oc, :])
                if oc % 2 == 1:
                    nc.sync.dma_start(
                        out=yv[prev["b"], :, oc - 1:oc + 1, prev["nsl"]],
                        in_=yt[:, oc - 1:oc + 1, :])

            # ================= main schedule =================
            S = {0: batch_tiles(0)}
            nxt = None
            prev = None
            _xq = [emit_A_load(S[0], 0, 0)]
            for i in range(1, NT):
                _xq.append(emit_A_load(S[0], 0, i))
            for i in range(NT):
                emit_A_tile(S[0], 0, i, _xq.pop(0))
                if i == 1:
                    nc.sync.dma_start(out=wo_sb, in_=wo_h)
                    nc.sync.dma_start(out=idn_sb, in_=idn_h)
                    nc.sync.dma_start(out=idb_sb, in_=idb_h)
            for b in range(BPC):
                if b + 1 < BPC:
                    S[b + 1] = batch_tiles(b + 1)

                def make_cur(bb, i):
                    nsl = slice(i * 512, (i + 1) * 512)
                    cur = {"nsl": nsl, "b": bb, "i": i, "ex": []}
                    cur["psOp"] = [
                        psum.tile([128, 2, 512], F32, tag="obmm", bufs=2,
                                  name=f"psOp{j}")
                        for j in range(2)
                    ]
                    # cross-batch safe: chunks 0/1 of phip are pooled by the
                    # A-phase of tiles 0/1, long since emitted
                    emit_L(S[bb], cur, 0)
                    emit_L(S[bb], cur, 1)
                    return cur

                for i in range(NT):
                    cur = nxt if nxt is not None else make_cur(b, i)
                    nxt = None
                    if i == 0:
                        # gT transposes for this batch (A fully emitted);
                        # interleaved into the k-loop below
                        emit_B_chunk(S[b], 0)
                        emit_B_chunk(S[b], 1)
                    emit_L(S[b], cur, 2)
                    emit_L(S[b], cur, 3)
                    if prev is not None:
                        # norms were interleaved with prev's k==7 bmm
                        emit_transp(prev)
                        emit_final_start(prev)
                    for k in range(8):
                        if i == 0 and k < 6:
                            emit_B_chunk(S[b], k + 2)
                        emit_bmm(S[b], cur, k, norm_tail=(k == 7))
                        if prev is not None and 1 <= k <= 4:
                            emit_final_oc(S[prev["b"]], prev, k - 1)
                        if k == 0:
                            emit_L(S[b], cur, 4)
                            emit_L(S[b], cur, 5)
                        if k == 2:
                            emit_L(S[b], cur, 6)
                            emit_L(S[b], cur, 7)
                        if k == 6:
                            if i + 1 < NT:
                                nxt = make_cur(b, i + 1)
                            elif b + 1 < BPC:
                                nxt = make_cur(b + 1, 0)

                    prev = cur
                    if b + 1 < BPC:
                        if i == 0:
                            _xq.append(emit_A_load(S[b + 1], b + 1, 0))
                        if i + 1 < NT:
                            _xq.append(emit_A_load(S[b + 1], b + 1, i + 1))
                        emit_A_tile(S[b + 1], b + 1, i, _xq.pop(0))
                # drain the last tile of the batch (norms already emitted
                # with its k==7 bmm)
                emit_transp(prev)
                emit_final_start(prev)
                for oc in range(4):
                    emit_final_oc(S[prev["b"]], prev, oc)

    nc.compile()
    return nc


_CACHE = {}


def _get_program():
    if "nc" not in _CACHE:
        _CACHE["nc"] = _build_program()
    return _CACHE["nc"]


def prepare_in_maps(inputs):
    x = np.ascontiguousarray(inputs["x"], dtype=np.float32).reshape(B, C, N)
    W_theta = np.asarray(inputs["W_theta"], dtype=np.float32)
    b_theta = np.asarray(inputs["b_theta"], dtype=np.float32)
    W_phi = np.asarray(inputs["W_phi"], dtype=np.float32)
    b_phi = np.asarray(inputs["b_phi"], dtype=np.float32)
    W_g = np.asarray(inputs["W_g"], dtype=np.float32)
    b_g = np.asarray(inputs["b_g"], dtype=np.float32)
    W_o = np.asarray(inputs["W_o"], dtype=np.float32)
    b_o = np.asarray(inputs["b_o"], dtype=np.float32)
    gamma = float(np.asarray(inputs["gamma"]).reshape(-1)[0])

    x16 = x.astype(np.float16)
    wtp16 = np.ascontiguousarray(
        np.concatenate([W_theta, W_phi], axis=0).T.reshape(4, 128, 128)
        .transpose(1, 0, 2)).astype(np.float16)
    wg16 = np.ascontiguousarray(
        W_g.T.reshape(4, 128, C2).transpose(1, 0, 2)).astype(np.float16)
    wo16 = np.ascontiguousarray(
        (gamma * W_o).T.reshape(2, 128, C).transpose(1, 0, 2)).astype(np.float16)
    idn16 = np.eye(128, dtype=np.float16)
    idnb = np.eye(128, dtype=np.float32).astype(ml_dtypes.bfloat16)

    biases = np.zeros((128, 8), np.float32)
    biases[0:64, 0] = b_theta
    biases[64:128, 0] = b_phi
    biases[:, 1] = b_g[0:128]
    biases[:, 2] = b_g[128:256]
    for oc in range(4):
        biases[:, 3 + oc] = gamma * b_o[oc * 128:(oc + 1) * 128]

    shared = {"wtp16": wtp16, "wg16": wg16, "wo16": wo16, "biases": biases,
              "idn16": idn16, "idnb": idnb}
    return [
        {"x16": np.ascontiguousarray(x16[c * BPC:(c + 1) * BPC]), **shared}
        for c in range(NCORES)
    ]


def kernel(**inputs) -> np.ndarray:
    in_maps = prepare_in_maps(inputs)
    nc = _get_program()
    res = run_bass_kernel_spmd(nc, in_maps, core_ids=list(range(NCORES)))
    y = np.concatenate(
        [np.asarray(r["y"]).astype(np.float32) for r in res.results], axis=0)
    return y.reshape(B, C, H, W)


if __name__ == "__main__":
    _get_program()
    print("program built OK")


# revision 4
# speedup vs baseline: 1.0088x; 1.0064x over previous
"""Trainium2 Bass kernel for PooledSelfAttention2d — 16-bit pipeline.

Derived from the f32r baseline schedule, with:
  - x loaded ONCE as f16 (8MB/core instead of 32MB): feeds the theta/phi
    conv, the g conv, and stays resident in SBUF for the residual add.
  - all matmul operands f16 (convs, logits, out conv) or bf16 (ex, gT —
    bf16 because ex = exp(logits - 60) spans e^[-55, 29], beyond f16 range).
  - transposes in 16-bit (1.0 cycles/row vs 1.5 for f32r).
  - y stored as f16 (8MB/core instead of 16MB), upcast on host.
  - g path pools PSUM directly (pool-first), relu+bias after the 4-way max.
  - exp/relu/copy work spread across Act, DVE, and GPSIMD; residual adds
    hit the DVE 2x mode (all-f16 operands).

Softmax uses the baseline's constant -60 shift (logits lie in [5, 89] for
this input distribution; bf16 has fp32's exponent range so no per-row max
is needed; softmax is invariant to the shift).
"""

import sys

if "/opt/trn_rl_repo" not in sys.path:
    sys.path.insert(0, "/opt/trn_rl_repo")

import contextlib

import ml_dtypes
import numpy as np

import concourse.bacc as bacc
import concourse.bass as bass
import concourse.tile as tile
from concourse import mybir
from concourse.bass_utils import run_bass_kernel_spmd

F32 = mybir.dt.float32
F16 = mybir.dt.float16
BF16 = mybir.dt.bfloat16
AF = mybir.ActivationFunctionType
ALU = mybir.AluOpType

B, C, H, W = 16, 512, 64, 64
N = H * W            # 4096 pixels
M = N // 4           # 1024 pooled pixels
K8 = C // 8          # 64  (theta/phi channels)
C2 = C // 2          # 256 (g channels)
NCORES = 8
BPC = B // NCORES    # batches per core
NT = N // 512        # n-tiles of 512 pixels
EXP_SHIFT = -60.0    # constant softmax shift


def _build_program():
    nc = bacc.Bacc("TRN2", target_bir_lowering=False, debug=False)

    x_h = nc.dram_tensor("x16", [BPC, C, N], F16, kind="ExternalInput").ap()
    wtp_h = nc.dram_tensor("wtp16", [128, 4, 128], F16, kind="ExternalInput").ap()
    wg_h = nc.dram_tensor("wg16", [128, 4, C2], F16, kind="ExternalInput").ap()
    wo_h = nc.dram_tensor("wo16", [128, 2, C], F16, kind="ExternalInput").ap()
    bia_h = nc.dram_tensor("biases", [128, 8], F32, kind="ExternalInput").ap()
    idn_h = nc.dram_tensor("idn16", [128, 128], F16, kind="ExternalInput").ap()
    idb_h = nc.dram_tensor("idnb", [128, 128], BF16, kind="ExternalInput").ap()
    y_h = nc.dram_tensor("y", [BPC, C, N], F16, kind="ExternalOutput").ap()

    xv = x_h.rearrange("b (cc p) n -> b p cc n", p=128)
    yv = y_h.rearrange("b (cc p) n -> b p cc n", p=128)

    with tile.TileContext(nc) as tc:
        with contextlib.ExitStack() as ctx:
            consts = ctx.enter_context(tc.tile_pool(name="consts", bufs=1))
            bpool = ctx.enter_context(tc.tile_pool(name="bpool", bufs=2))
            xpool = ctx.enter_context(tc.tile_pool(name="xpool", bufs=1))
            work = ctx.enter_context(tc.tile_pool(name="work", bufs=2))
            psum = ctx.enter_context(tc.tile_pool(name="psum", bufs=1, space="PSUM"))

            # ---- weights first (small; the serial DMA line serves them
            # before xr0 so the first conv starts sooner), then x ----
            wtp_sb = consts.tile([128, 4, 128], F16, tag="wtp")
            nc.sync.dma_start(out=wtp_sb, in_=wtp_h)
            xr0 = xpool.tile([128, 4, 512], F16, tag="xr", bufs=NT * 2 + 1)
            nc.gpsimd.dma_start(out=xr0, in_=xv[0, :, :, 0:512])
            wg_sb = consts.tile([128, 4, C2], F16, tag="wg")
            nc.sync.dma_start(out=wg_sb, in_=wg_h)
            bia_sb = consts.tile([128, 8], F32, tag="bia")
            nc.sync.dma_start(out=bia_sb, in_=bia_h)
            wo_sb = consts.tile([128, 2, C], F16, tag="wo")
            idn_sb = consts.tile([128, 128], F16, tag="idn")
            idb_sb = consts.tile([128, 128], BF16, tag="idb")
            shift_sb = consts.tile([128, 1], F32, tag="shift")
            nc.vector.memset(shift_sb, EXP_SHIFT)
            onez_sb = consts.tile([128, 2], BF16, tag="onez")
            nc.vector.memset(onez_sb, 0.0)
            nc.vector.memset(onez_sb[:, 0:1], 1.0)

            # per-batch persistent tiles
            def batch_tiles(b):
                tp = bpool.tile([128, N], F16, tag="tp", name=f"tp{b}")
                phip = bpool.tile([64, M], F16, tag="phip", name=f"phip{b}")
                gp = bpool.tile([128, 2, M], BF16, tag="gp", name=f"gp{b}")
                gT = bpool.tile([128, 8, C2 + 2], BF16, tag="gT", name=f"gT{b}")
                onez_b = bass.AP(
                    tensor=onez_sb.tensor,
                    offset=onez_sb.offset,
                    ap=[list(onez_sb.ap[0]), [0, 8], list(onez_sb.ap[1])],
                )
                nc.vector.tensor_copy(gT[:, :, C2:C2 + 2], onez_b)
                return dict(tp=tp, phip=phip, gp=gp, gT=gT, xr=[None] * NT)

            def emit_A_load(S, b, i):
                nsl = slice(i * 512, (i + 1) * 512)
                if b == 0 and i == 0:
                    S["xr"][i] = xr0
                    return xr0
                xr = xpool.tile([128, 4, 512], F16, tag="xr", bufs=NT * 2 + 1,
                                name=f"xr{b}_{i}")
                eng = nc.gpsimd if i % 2 == 0 else nc.sync
                eng.dma_start(out=xr, in_=xv[b, :, :, nsl])
                S["xr"][i] = xr
                return xr

            def emit_A_tile(S, b, i, xr):
                nsl = slice(i * 512, (i + 1) * 512)
                msl = slice(i * 128, (i + 1) * 128)
                psTP = psum.tile([128, 512], F32, tag="mmL", bufs=4)
                for cc in range(4):
                    nc.tensor.matmul(psTP, wtp_sb[:, cc, :], xr[:, cc, :],
                                     start=(cc == 0), stop=(cc == 3))
                # theta+phi relu+bias evac (Act)
                nc.scalar.activation(S["tp"][:, nsl], psTP, AF.Relu,
                                     bias=bia_sb[:, 0:1])
                for oc in range(2):
                    psG = psum.tile([128, 512], F32, tag="mmL", bufs=4,
                                    name=f"psG{oc}")
                    for cc in range(4):
                        nc.tensor.matmul(
                            psG, wg_sb[:, cc, oc * 128:(oc + 1) * 128],
                            xr[:, cc, :], start=(cc == 0), stop=(cc == 3))
                    # relu+bias evac (Act; gpsimd cannot read PSUM), then
                    # 2x2 maxpool on SBUF (gpsimd)
                    gf = work.tile([128, 512], BF16, tag="gf", bufs=4,
                                   name=f"gf{oc}")
                    nc.scalar.activation(gf, psG, AF.Relu,
                                         bias=bia_sb[:, 1 + oc:2 + oc])
                    gv = gf.rearrange("p (a two) -> p a two", two=2)
                    gw = work.tile([128, 256], BF16, tag="gw", bufs=4,
                                   name=f"gw{oc}")
                    nc.vector.tensor_max(gw, gv[:, :, 0], gv[:, :, 1])
                    gw2 = gw.rearrange("p (h two w) -> p h two w", two=2, w=32)
                    gdst = S["gp"][:, oc, msl].rearrange("p (h w) -> p h w", w=32)
                    nc.vector.tensor_max(gdst, gw2[:, :, 0, :], gw2[:, :, 1, :])
                # phi pool (gpsimd, SBUF) from tp rows 64:128
                phv = S["tp"][64:128, nsl].rearrange("p (a two) -> p a two",
                                                     two=2)
                phw = work.tile([64, 256], F16, tag="phw", bufs=4)
                nc.vector.tensor_max(phw, phv[:, :, 0], phv[:, :, 1])
                phw2 = phw.rearrange("p (h two w) -> p h two w", two=2, w=32)
                pdst = S["phip"][:, msl].rearrange("p (h w) -> p h w", w=32)
                nc.vector.tensor_max(pdst, phw2[:, :, 0, :], phw2[:, :, 1, :])

            def emit_B_chunk(S, mi):
                """transpose pooled g chunk mi to gT (bf16)."""
                msl = slice(mi * 128, (mi + 1) * 128)
                psT = psum.tile([128, C2], BF16, tag="mmL", bufs=4,
                                name=f"psT{mi}")
                nc.tensor.transpose(psT[:, 0:128], S["gp"][:, 0, msl], idb_sb)
                nc.tensor.transpose(psT[:, 128:256], S["gp"][:, 1, msl], idb_sb)
                nc.scalar.copy(S["gT"][:, mi, 0:C2], psT)

            def emit_L(S, cur, k):
                """logits for m-chunk k + exp -> bf16."""
                psL = psum.tile([128, 512], F32, tag="mmL", bufs=4)
                nc.tensor.matmul(psL, S["phip"][:, k * 128:(k + 1) * 128],
                                 S["tp"][0:64, cur["nsl"]], start=True,
                                 stop=True)
                ex = work.tile([128, 512], BF16, tag="exp", bufs=6)
                nc.scalar.activation(ex, psL, AF.Exp, bias=shift_sb)
                cur["ex"].append(ex)

            def emit_bmm(S, cur, k, norm_tail=False):
                for ns in range(4):
                    ssl = slice(ns * 128, (ns + 1) * 128)
                    nc.tensor.matmul(
                        cur["psOp"][ns // 2][:, ns % 2, 0:C2 + 2],
                        cur["ex"][k][:, ssl], S["gT"][:, k, :],
                        start=(k == 0), stop=(k == 7))
                    if norm_tail:
                        emit_norm_ns(cur, ns)

            def emit_norm_ns(prev, ns):
                """normalize one n-sub of tile i-1 (DVE only) -> f16."""
                psO = prev["psOp"][ns // 2][:, ns % 2, :]
                rec = work.tile([128, 1], F32, tag="rec", bufs=8)
                nc.vector.reciprocal(rec, psO[:, C2:C2 + 1])
                onc = work.tile([128, C2], F16, tag="onc", bufs=8,
                                name=f"onc{ns}")
                nc.vector.tensor_scalar_mul(onc, psO[:, 0:C2], rec)
                prev.setdefault("onc", []).append(onc)

            def emit_transp(prev):
                """transpose tile i-1 back to channel-major (PE, f16)."""
                prev["pst"] = [
                    psum.tile([128, 512], F16, tag="mmL", bufs=4,
                              name=f"pst{c2}")
                    for c2 in range(2)
                ]
                for ns in range(4):
                    ssl = slice(ns * 128, (ns + 1) * 128)
                    onc = prev["onc"][ns]
                    for c2 in range(2):
                        nc.tensor.transpose(
                            prev["pst"][c2][:, ssl],
                            onc[:, c2 * 128:(c2 + 1) * 128], idn_sb)

            def emit_final_start(prev):
                """evacuate transposed o to SBUF (tile i-1), f16."""
                ocm = [
                    work.tile([128, 512], F16, tag="ocm", bufs=4,
                              name=f"ocm{j}")
                    for j in range(2)
                ]
                nc.vector.tensor_copy(ocm[0], prev["pst"][0])
                nc.vector.tensor_copy(ocm[1], prev["pst"][1])
                prev["ocm"] = ocm
                prev["yt"] = work.tile([128, 4, 512], F16, tag="y", bufs=3,
                                       name="yt")

            def emit_final_oc(S, prev, oc, drain=False):
                """one output-conv chunk + relu+bias + residual (+ store)."""
                ocm, yt = prev["ocm"], prev["yt"]
                psY = psum.tile([128, 512], F32, tag="mmL", bufs=4)
                nc.tensor.matmul(psY, wo_sb[:, 0, oc * 128:(oc + 1) * 128],
                                 ocm[0], start=True, stop=False)
                nc.tensor.matmul(psY, wo_sb[:, 1, oc * 128:(oc + 1) * 128],
                                 ocm[1], start=False, stop=True)
                # y = relu(conv + b) + x   (relu+bias split across engines)
                if drain or oc % 2 == 1:
                    nc.scalar.activation(yt[:, oc, :], psY, AF.Relu,
                                         bias=bia_sb[:, 3 + oc:4 + oc])
                else:
                    nc.vector.tensor_scalar(yt[:, oc, :], psY,
                                            bia_sb[:, 3 + oc:4 + oc], 0.0,
                                            ALU.add, ALU.max)
                xr = S["xr"][prev["i"]]
                nc.vector.tensor_add(yt[:, oc, :], yt[:, oc, :], xr[:, oc, :])
                if oc % 2 == 1:
                    nc.sync.dma_start(
                        out=yv[prev["b"], :, oc - 1:oc + 1, prev["nsl"]],
                        in_=yt[:, oc - 1:oc + 1, :])

            # ================= main schedule =================
            S = {0: batch_tiles(0)}
            nxt = None
            prev = None
            _xq = [emit_A_load(S[0], 0, 0)]
            for i in range(1, NT):
                _xq.append(emit_A_load(S[0], 0, i))
            for i in range(NT):
                emit_A_tile(S[0], 0, i, _xq.pop(0))
                if i == 1:
                    nc.sync.dma_start(out=wo_sb, in_=wo_h)
                    nc.sync.dma_start(out=idn_sb, in_=idn_h)
                    nc.sync.dma_start(out=idb_sb, in_=idb_h)
            for b in range(BPC):
                if b + 1 < BPC:
                    S[b + 1] = batch_tiles(b + 1)

                def make_cur(bb, i, nl=2):
                    nsl = slice(i * 512, (i + 1) * 512)
                    cur = {"nsl": nsl, "b": bb, "i": i, "ex": []}
                    cur["psOp"] = [
                        psum.tile([128, 2, 512], F32, tag="obmm", bufs=2,
                                  name=f"psOp{j}")
                        for j in range(2)
                    ]
                    # cross-batch safe: chunks 0/1 of phip are pooled by the
                    # A-phase of tiles 0/1, long since emitted
                    for k in range(nl):
                        emit_L(S[bb], cur, k)
                    return cur

                for i in range(NT):
                    cur = nxt if nxt is not None else make_cur(b, i)
                    nxt = None
                    if i == 0:
                        # gT transposes for this batch (A fully emitted);
                        # interleaved into the k-loop below
                        emit_B_chunk(S[b], 0)
                        emit_B_chunk(S[b], 1)
                    emit_L(S[b], cur, 2)
                    emit_L(S[b], cur, 3)
                    if prev is not None:
                        # norms were interleaved with prev's k==7 bmm
                        emit_transp(prev)
                        emit_final_start(prev)
                    for k in range(8):
                        if i == 0 and k < 6:
                            emit_B_chunk(S[b], k + 2)
                        emit_bmm(S[b], cur, k, norm_tail=(k == 7))
                        if prev is not None and 1 <= k <= 4:
                            emit_final_oc(S[prev["b"]], prev, k - 1)
                        if k == 0:
                            emit_L(S[b], cur, 4)
                            emit_L(S[b], cur, 5)
                        if k == 2:
                            emit_L(S[b], cur, 6)
                            emit_L(S[b], cur, 7)
                        if k == 6:
                            if i + 1 < NT:
                                nxt = make_cur(b, i + 1)
                            elif b + 1 < BPC:
                                nxt = make_cur(b + 1, 0)

                    prev = cur
                    if b + 1 < BPC:
                        if i == 0:
                            _xq.append(emit_A_load(S[b + 1], b + 1, 0))
                        if i + 1 < NT:
                            _xq.append(emit_A_load(S[b + 1], b + 1, i + 1))
                        emit_A_tile(S[b + 1], b + 1, i, _xq.pop(0))
                # drain the last tile of the batch (norms already emitted
                # with its k==7 bmm)
                emit_transp(prev)
                emit_final_start(prev)
                for oc in range(4):
                    emit_final_oc(S[prev["b"]], prev, oc)

    nc.compile()
    return nc


_CACHE = {}


def _get_program():
    if "nc" not in _CACHE:
        _CACHE["nc"] = _build_program()
    return _CACHE["nc"]


def prepare_in_maps(inputs):
    x = np.ascontiguousarray(inputs["x"], dtype=np.float32).reshape(B, C, N)
    W_theta = np.asarray(inputs["W_theta"], dtype=np.float32)
    b_theta = np.asarray(inputs["b_theta"], dtype=np.float32)
    W_phi = np.asarray(inputs["W_phi"], dtype=np.float32)
    b_phi = np.asarray(inputs["b_phi"], dtype=np.float32)
    W_g = np.asarray(inputs["W_g"], dtype=np.float32)
    b_g = np.asarray(inputs["b_g"], dtype=np.float32)
    W_o = np.asarray(inputs["W_o"], dtype=np.float32)
    b_o = np.asarray(inputs["b_o"], dtype=np.float32)
    gamma = float(np.asarray(inputs["gamma"]).reshape(-1)[0])

    x16 = x.astype(np.float16)
    wtp16 = np.ascontiguousarray(
        np.concatenate([W_theta, W_phi], axis=0).T.reshape(4, 128, 128)
        .transpose(1, 0, 2)).astype(np.float16)
    wg16 = np.ascontiguousarray(
        W_g.T.reshape(4, 128, C2).transpose(1, 0, 2)).astype(np.float16)
    wo16 = np.ascontiguousarray(
        (gamma * W_o).T.reshape(2, 128, C).transpose(1, 0, 2)).astype(np.float16)
    idn16 = np.eye(128, dtype=np.float16)
    idnb = np.eye(128, dtype=np.float32).astype(ml_dtypes.bfloat16)

    biases = np.zeros((128, 8), np.float32)
    biases[0:64, 0] = b_theta
    biases[64:128, 0] = b_phi
    biases[:, 1] = b_g[0:128]
    biases[:, 2] = b_g[128:256]
    for oc in range(4):
        biases[:, 3 + oc] = gamma * b_o[oc * 128:(oc + 1) * 128]

    shared = {"wtp16": wtp16, "wg16": wg16, "wo16": wo16, "biases": biases,
              "idn16": idn16, "idnb": idnb}
    return [
        {"x16": np.ascontiguousarray(x16[c * BPC:(c + 1) * BPC]), **shared}
        for c in range(NCORES)
    ]


def kernel(**inputs) -> np.ndarray:
    in_maps = prepare_in_maps(inputs)
    nc = _get_program()
    res = run_bass_kernel_spmd(nc, in_maps, core_ids=list(range(NCORES)))
    y = np.concatenate(
        [np.asarray(r["y"]).astype(np.float32) for r in res.results], axis=0)
    return y.reshape(B, C, H, W)


if __name__ == "__main__":
    _get_program()
    print("program built OK")


# revision 5
# speedup vs baseline: 1.0215x; 1.0126x over previous
"""Trainium2 Bass kernel for PooledSelfAttention2d — 16-bit pipeline.

Derived from the f32r baseline schedule, with:
  - x loaded ONCE as f16 (8MB/core instead of 32MB): feeds the theta/phi
    conv, the g conv, and stays resident in SBUF for the residual add.
  - all matmul operands f16 (convs, logits, out conv) or bf16 (ex, gT —
    bf16 because ex = exp(logits - 60) spans e^[-55, 29], beyond f16 range).
  - transposes in 16-bit (1.0 cycles/row vs 1.5 for f32r).
  - y stored as f16 (8MB/core instead of 16MB), upcast on host.
  - g path pools PSUM directly (pool-first), relu+bias after the 4-way max.
  - exp/relu/copy work spread across Act, DVE, and GPSIMD; residual adds
    hit the DVE 2x mode (all-f16 operands).

Softmax uses the baseline's constant -60 shift (logits lie in [5, 89] for
this input distribution; bf16 has fp32's exponent range so no per-row max
is needed; softmax is invariant to the shift).
"""

import sys

if "/opt/trn_rl_repo" not in sys.path:
    sys.path.insert(0, "/opt/trn_rl_repo")

import contextlib

import ml_dtypes
import numpy as np

import concourse.bacc as bacc
import concourse.bass as bass
import concourse.tile as tile
from concourse import mybir
from concourse.bass_utils import run_bass_kernel_spmd

F32 = mybir.dt.float32
F16 = mybir.dt.float16
BF16 = mybir.dt.bfloat16
AF = mybir.ActivationFunctionType
ALU = mybir.AluOpType

B, C, H, W = 16, 512, 64, 64
N = H * W            # 4096 pixels
M = N // 4           # 1024 pooled pixels
K8 = C // 8          # 64  (theta/phi channels)
C2 = C // 2          # 256 (g channels)
NCORES = 8
BPC = B // NCORES    # batches per core
NT = N // 512        # n-tiles of 512 pixels
EXP_SHIFT = -60.0    # constant softmax shift


def _build_program():
    nc = bacc.Bacc("TRN2", target_bir_lowering=False, debug=False)

    x_h = nc.dram_tensor("x16", [BPC, C, N], F16, kind="ExternalInput").ap()
    wtp_h = nc.dram_tensor("wtp16", [128, 4, 128], F16, kind="ExternalInput").ap()
    wg_h = nc.dram_tensor("wg16", [128, 4, C2], F16, kind="ExternalInput").ap()
    wo_h = nc.dram_tensor("wo16", [128, 2, C], F16, kind="ExternalInput").ap()
    bia_h = nc.dram_tensor("biases", [128, 8], F32, kind="ExternalInput").ap()
    idn_h = nc.dram_tensor("idn16", [128, 128], F16, kind="ExternalInput").ap()
    idb_h = nc.dram_tensor("idnb", [128, 128], BF16, kind="ExternalInput").ap()
    y_h = nc.dram_tensor("y", [BPC, C, N], F16, kind="ExternalOutput").ap()

    xv = x_h.rearrange("b (cc p) n -> b p cc n", p=128)
    yv = y_h.rearrange("b (cc p) n -> b p cc n", p=128)

    with tile.TileContext(nc) as tc:
        with contextlib.ExitStack() as ctx:
            consts = ctx.enter_context(tc.tile_pool(name="consts", bufs=1))
            bpool = ctx.enter_context(tc.tile_pool(name="bpool", bufs=2))
            xpool = ctx.enter_context(tc.tile_pool(name="xpool", bufs=1))
            work = ctx.enter_context(tc.tile_pool(name="work", bufs=2))
            psum = ctx.enter_context(tc.tile_pool(name="psum", bufs=1, space="PSUM"))

            # ---- weights first (small; the serial DMA line serves them
            # before xr0 so the first conv starts sooner), then x ----
            wtp_sb = consts.tile([128, 4, 128], F16, tag="wtp")
            nc.sync.dma_start(out=wtp_sb, in_=wtp_h)
            xr0 = xpool.tile([128, 4, 512], F16, tag="xr", bufs=NT * 2 + 1)
            nc.gpsimd.dma_start(out=xr0, in_=xv[0, :, :, 0:512])
            wg_sb = consts.tile([128, 4, C2], F16, tag="wg")
            nc.sync.dma_start(out=wg_sb, in_=wg_h)
            bia_sb = consts.tile([128, 8], F32, tag="bia")
            nc.sync.dma_start(out=bia_sb, in_=bia_h)
            wo_sb = consts.tile([128, 2, C], F16, tag="wo")
            idn_sb = consts.tile([128, 128], F16, tag="idn")
            idb_sb = consts.tile([128, 128], BF16, tag="idb")
            shift_sb = consts.tile([128, 1], F32, tag="shift")
            nc.vector.memset(shift_sb, EXP_SHIFT)
            onez_sb = consts.tile([128, 2], BF16, tag="onez")
            nc.vector.memset(onez_sb, 0.0)
            nc.vector.memset(onez_sb[:, 0:1], 1.0)

            # per-batch persistent tiles
            def batch_tiles(b):
                tp = bpool.tile([128, N], F16, tag="tp", name=f"tp{b}")
                phip = bpool.tile([64, M], F16, tag="phip", name=f"phip{b}")
                gp = bpool.tile([128, 2, M], BF16, tag="gp", name=f"gp{b}")
                gT = bpool.tile([128, 8, C2 + 2], BF16, tag="gT", name=f"gT{b}")
                onez_b = bass.AP(
                    tensor=onez_sb.tensor,
                    offset=onez_sb.offset,
                    ap=[list(onez_sb.ap[0]), [0, 8], list(onez_sb.ap[1])],
                )
                nc.vector.tensor_copy(gT[:, :, C2:C2 + 2], onez_b)
                return dict(tp=tp, phip=phip, gp=gp, gT=gT, xr=[None] * NT)

            def emit_A_load(S, b, i):
                nsl = slice(i * 512, (i + 1) * 512)
                if b == 0 and i == 0:
                    S["xr"][i] = xr0
                    return xr0
                xr = xpool.tile([128, 4, 512], F16, tag="xr", bufs=NT * 2 + 1,
                                name=f"xr{b}_{i}")
                eng = nc.gpsimd if i % 2 == 0 else nc.sync
                eng.dma_start(out=xr, in_=xv[b, :, :, nsl])
                S["xr"][i] = xr
                return xr

            def emit_A_tile(S, b, i, xr):
                nsl = slice(i * 512, (i + 1) * 512)
                msl = slice(i * 128, (i + 1) * 128)
                psTP = psum.tile([128, 512], F32, tag="mmL", bufs=4)
                for cc in range(4):
                    nc.tensor.matmul(psTP, wtp_sb[:, cc, :], xr[:, cc, :],
                                     start=(cc == 0), stop=(cc == 3))
                # theta+phi relu+bias evac (Act)
                nc.scalar.activation(S["tp"][:, nsl], psTP, AF.Relu,
                                     bias=bia_sb[:, 0:1])
                for oc in range(2):
                    psG = psum.tile([128, 512], F32, tag="mmL", bufs=4,
                                    name=f"psG{oc}")
                    for cc in range(4):
                        nc.tensor.matmul(
                            psG, wg_sb[:, cc, oc * 128:(oc + 1) * 128],
                            xr[:, cc, :], start=(cc == 0), stop=(cc == 3))
                    # relu+bias evac (Act; gpsimd cannot read PSUM), then
                    # 2x2 maxpool on SBUF (gpsimd)
                    gf = work.tile([128, 512], BF16, tag="gf", bufs=4,
                                   name=f"gf{oc}")
                    nc.scalar.activation(gf, psG, AF.Relu,
                                         bias=bia_sb[:, 1 + oc:2 + oc])
                    gv = gf.rearrange("p (a two) -> p a two", two=2)
                    gw = work.tile([128, 256], BF16, tag="gw", bufs=4,
                                   name=f"gw{oc}")
                    nc.vector.tensor_max(gw, gv[:, :, 0], gv[:, :, 1])
                    gw2 = gw.rearrange("p (h two w) -> p h two w", two=2, w=32)
                    gdst = S["gp"][:, oc, msl].rearrange("p (h w) -> p h w", w=32)
                    nc.vector.tensor_max(gdst, gw2[:, :, 0, :], gw2[:, :, 1, :])
                # phi pool (gpsimd, SBUF) from tp rows 64:128
                phv = S["tp"][64:128, nsl].rearrange("p (a two) -> p a two",
                                                     two=2)
                phw = work.tile([64, 256], F16, tag="phw", bufs=4)
                nc.vector.tensor_max(phw, phv[:, :, 0], phv[:, :, 1])
                phw2 = phw.rearrange("p (h two w) -> p h two w", two=2, w=32)
                pdst = S["phip"][:, msl].rearrange("p (h w) -> p h w", w=32)
                nc.vector.tensor_max(pdst, phw2[:, :, 0, :], phw2[:, :, 1, :])

            def emit_B_chunk(S, mi):
                """transpose pooled g chunk mi to gT (bf16)."""
                msl = slice(mi * 128, (mi + 1) * 128)
                psT = psum.tile([128, C2], BF16, tag="mmL", bufs=4,
                                name=f"psT{mi}")
                nc.tensor.transpose(psT[:, 0:128], S["gp"][:, 0, msl], idb_sb)
                nc.tensor.transpose(psT[:, 128:256], S["gp"][:, 1, msl], idb_sb)
                nc.scalar.copy(S["gT"][:, mi, 0:C2], psT)

            def emit_L(S, cur, k):
                """logits for m-chunk k + exp -> bf16."""
                psL = psum.tile([128, 512], F32, tag="mmL", bufs=4)
                nc.tensor.matmul(psL, S["phip"][:, k * 128:(k + 1) * 128],
                                 S["tp"][0:64, cur["nsl"]], start=True,
                                 stop=True)
                ex = work.tile([128, 512], BF16, tag="exp", bufs=6)
                nc.scalar.activation(ex, psL, AF.Exp, bias=shift_sb)
                cur["ex"].append(ex)

            def emit_bmm(S, cur, k, norm_tail=False):
                for ns in range(4):
                    ssl = slice(ns * 128, (ns + 1) * 128)
                    nc.tensor.matmul(
                        cur["psOp"][ns // 2][:, ns % 2, 0:C2 + 1],
                        cur["ex"][k][:, ssl], S["gT"][:, k, 0:C2 + 1],
                        start=(k == 0), stop=(k == 7))
                    if norm_tail:
                        emit_norm_ns(cur, ns)

            def emit_norm_ns(prev, ns):
                """normalize one n-sub of tile i-1 (DVE only) -> f16."""
                psO = prev["psOp"][ns // 2][:, ns % 2, :]
                rec = work.tile([128, 1], F32, tag="rec", bufs=8)
                nc.vector.reciprocal(rec, psO[:, C2:C2 + 1])
                onc = work.tile([128, C2], F16, tag="onc", bufs=8,
                                name=f"onc{ns}")
                nc.vector.tensor_scalar_mul(onc, psO[:, 0:C2], rec)
                prev.setdefault("onc", []).append(onc)

            def emit_transp(prev):
                """transpose tile i-1 back to channel-major (PE, f16)."""
                prev["pst"] = [
                    psum.tile([128, 512], F16, tag="mmL", bufs=4,
                              name=f"pst{c2}")
                    for c2 in range(2)
                ]
                for ns in range(4):
                    ssl = slice(ns * 128, (ns + 1) * 128)
                    onc = prev["onc"][ns]
                    for c2 in range(2):
                        nc.tensor.transpose(
                            prev["pst"][c2][:, ssl],
                            onc[:, c2 * 128:(c2 + 1) * 128], idn_sb)

            def emit_final_start(prev):
                """evacuate transposed o to SBUF (tile i-1), f16."""
                ocm = [
                    work.tile([128, 512], F16, tag="ocm", bufs=4,
                              name=f"ocm{j}")
                    for j in range(2)
                ]
                nc.vector.tensor_copy(ocm[0], prev["pst"][0])
                nc.vector.tensor_copy(ocm[1], prev["pst"][1])
                prev["ocm"] = ocm
                prev["yt"] = work.tile([128, 4, 512], F16, tag="y", bufs=3,
                                       name="yt")

            def emit_final_oc(S, prev, oc, drain=False):
                """one output-conv chunk + relu+bias + residual (+ store)."""
                ocm, yt = prev["ocm"], prev["yt"]
                psY = psum.tile([128, 512], F32, tag="mmL", bufs=4)
                nc.tensor.matmul(psY, wo_sb[:, 0, oc * 128:(oc + 1) * 128],
                                 ocm[0], start=True, stop=False)
                nc.tensor.matmul(psY, wo_sb[:, 1, oc * 128:(oc + 1) * 128],
                                 ocm[1], start=False, stop=True)
                # y = relu(conv + b) + x   (relu+bias split across engines)
                if drain or oc % 2 == 1:
                    nc.scalar.activation(yt[:, oc, :], psY, AF.Relu,
                                         bias=bia_sb[:, 3 + oc:4 + oc])
                else:
                    nc.vector.tensor_scalar(yt[:, oc, :], psY,
                                            bia_sb[:, 3 + oc:4 + oc], 0.0,
                                            ALU.add, ALU.max)
                xr = S["xr"][prev["i"]]
                nc.vector.tensor_add(yt[:, oc, :], yt[:, oc, :], xr[:, oc, :])
                if oc % 2 == 1:
                    nc.sync.dma_start(
                        out=yv[prev["b"], :, oc - 1:oc + 1, prev["nsl"]],
                        in_=yt[:, oc - 1:oc + 1, :])

            # ================= main schedule =================
            S = {0: batch_tiles(0)}
            nxt = None
            prev = None
            _xq = [emit_A_load(S[0], 0, 0)]
            for i in range(1, NT):
                _xq.append(emit_A_load(S[0], 0, i))
            for i in range(NT):
                emit_A_tile(S[0], 0, i, _xq.pop(0))
                if i == 1:
                    nc.sync.dma_start(out=wo_sb, in_=wo_h)
                    nc.sync.dma_start(out=idn_sb, in_=idn_h)
                    nc.sync.dma_start(out=idb_sb, in_=idb_h)
            for b in range(BPC):
                if b + 1 < BPC:
                    S[b + 1] = batch_tiles(b + 1)

                def make_cur(bb, i, nl=2):
                    nsl = slice(i * 512, (i + 1) * 512)
                    cur = {"nsl": nsl, "b": bb, "i": i, "ex": []}
                    cur["psOp"] = [
                        psum.tile([128, 2, 512], F32, tag="obmm", bufs=2,
                                  name=f"psOp{j}")
                        for j in range(2)
                    ]
                    # cross-batch safe: chunks 0/1 of phip are pooled by the
                    # A-phase of tiles 0/1, long since emitted
                    for k in range(nl):
                        emit_L(S[bb], cur, k)
                    return cur

                for i in range(NT):
                    cur = nxt if nxt is not None else make_cur(b, i)
                    nxt = None
                    if i == 0:
                        # gT transposes for this batch (A fully emitted);
                        # interleaved into the k-loop below
                        emit_B_chunk(S[b], 0)
                        emit_B_chunk(S[b], 1)
                    emit_L(S[b], cur, 2)
                    emit_L(S[b], cur, 3)
                    for k in range(8):
                        if i == 0 and k < 6:
                            emit_B_chunk(S[b], k + 2)
                        emit_bmm(S[b], cur, k, norm_tail=(k == 7))
                        if prev is not None and k == 0:
                            # norms were interleaved with prev's k==7 bmm
                            emit_transp(prev)
                            emit_final_start(prev)
                        if prev is not None and 1 <= k <= 4:
                            emit_final_oc(S[prev["b"]], prev, k - 1)
                        if k == 0:
                            emit_L(S[b], cur, 4)
                            emit_L(S[b], cur, 5)
                        if k == 2:
                            emit_L(S[b], cur, 6)
                            emit_L(S[b], cur, 7)
                        if k == 6:
                            if i + 1 < NT:
                                nxt = make_cur(b, i + 1)
                            elif b + 1 < BPC:
                                nxt = make_cur(b + 1, 0)

                    prev = cur
                    if b + 1 < BPC:
                        if i == 0:
                            _xq.append(emit_A_load(S[b + 1], b + 1, 0))
                        if i + 1 < NT:
                            _xq.append(emit_A_load(S[b + 1], b + 1, i + 1))
                        emit_A_tile(S[b + 1], b + 1, i, _xq.pop(0))
                # drain the last tile of the batch (norms already emitted
                # with its k==7 bmm)
                emit_transp(prev)
                emit_final_start(prev)
                for oc in range(4):
                    emit_final_oc(S[prev["b"]], prev, oc)

    nc.compile()
    return nc


_CACHE = {}


def _get_program():
    if "nc" not in _CACHE:
        _CACHE["nc"] = _build_program()
    return _CACHE["nc"]


def prepare_in_maps(inputs):
    x = np.ascontiguousarray(inputs["x"], dtype=np.float32).reshape(B, C, N)
    W_theta = np.asarray(inputs["W_theta"], dtype=np.float32)
    b_theta = np.asarray(inputs["b_theta"], dtype=np.float32)
    W_phi = np.asarray(inputs["W_phi"], dtype=np.float32)
    b_phi = np.asarray(inputs["b_phi"], dtype=np.float32)
    W_g = np.asarray(inputs["W_g"], dtype=np.float32)
    b_g = np.asarray(inputs["b_g"], dtype=np.float32)
    W_o = np.asarray(inputs["W_o"], dtype=np.float32)
    b_o = np.asarray(inputs["b_o"], dtype=np.float32)
    gamma = float(np.asarray(inputs["gamma"]).reshape(-1)[0])

    x16 = x.astype(np.float16)
    wtp16 = np.ascontiguousarray(
        np.concatenate([W_theta, W_phi], axis=0).T.reshape(4, 128, 128)
        .transpose(1, 0, 2)).astype(np.float16)
    wg16 = np.ascontiguousarray(
        W_g.T.reshape(4, 128, C2).transpose(1, 0, 2)).astype(np.float16)
    wo16 = np.ascontiguousarray(
        (gamma * W_o).T.reshape(2, 128, C).transpose(1, 0, 2)).astype(np.float16)
    idn16 = np.eye(128, dtype=np.float16)
    idnb = np.eye(128, dtype=np.float32).astype(ml_dtypes.bfloat16)

    biases = np.zeros((128, 8), np.float32)
    biases[0:64, 0] = b_theta
    biases[64:128, 0] = b_phi
    biases[:, 1] = b_g[0:128]
    biases[:, 2] = b_g[128:256]
    for oc in range(4):
        biases[:, 3 + oc] = gamma * b_o[oc * 128:(oc + 1) * 128]

    shared = {"wtp16": wtp16, "wg16": wg16, "wo16": wo16, "biases": biases,
              "idn16": idn16, "idnb": idnb}
    return [
        {"x16": np.ascontiguousarray(x16[c * BPC:(c + 1) * BPC]), **shared}
        for c in range(NCORES)
    ]


def kernel(**inputs) -> np.ndarray:
    in_maps = prepare_in_maps(inputs)
    nc = _get_program()
    res = run_bass_kernel_spmd(nc, in_maps, core_ids=list(range(NCORES)))
    y = np.concatenate(
        [np.asarray(r["y"]).astype(np.float32) for r in res.results], axis=0)
    return y.reshape(B, C, H, W)


if __name__ == "__main__":
    _get_program()
    print("program built OK")
